# Initial kernel scaffold
#
"""Multi-head attention (16 heads, DM=1024, DK=DV=64, S=2048, B=2, causal)
tensor-parallel over heads on 8 NeuronCores (2 heads per core).

Host-side marshalling:
  - Activations are pre-transposed to XT[B, DM, S] so the device matmuls
    contract over DM on the partition dimension with natural layouts.
  - Per-core weights: WQ/WK/WV head pair stacked on columns -> [DM, 128];
    WO rows for the head pair -> [128, DM].
  - Each core computes its 2 heads end-to-end plus the WO partial
    projection; host sums the 8 partial outputs.

Device layouts (per batch b):
  qT, kT  [128(2 heads x dk), S]   = (W.T @ XT) via lhsT=W chunk, rhs=XT chunk
  v       [S, 128(2 heads x dv)]   interleaved with ones cols for row-sums
  ST      [s_k 128, s_q 512]       = kT_tile.T-style scores, transposed
  pT      = exp(ST/8) (no max subtraction needed: |scores/8| <= ~2)
  oT      [65, s_q 512] accum      rows 0:64 = unnormalized head out, row 64 = rowsum
  y       [s 128, dm 512]          = oT.T @ WO_local, accumulated over both heads via K=128
"""

import numpy as np

S, B, DM, DK, DV, H = 2048, 2, 1024, 64, 64, 16
NCORES = 8
HEADS_PER_CORE = H // NCORES  # 2
SCALE = 1.0 / np.sqrt(DK)  # 1/8

F32 = None  # filled lazily (mybir import)

_CACHE = {}


def build_nc(split_waits=True):
    import concourse.bass as bass
    import concourse.tile as tile
    from concourse import mybir

    f32 = mybir.dt.float32
    bf16 = mybir.dt.bfloat16
    nc = bass.Bass()

    xtq = nc.dram_tensor("xtq", [B, DM, S], bf16, kind="ExternalInput")
    xtk = nc.dram_tensor("xtk", [B, DM, S], bf16, kind="ExternalInput")
    xtv = nc.dram_tensor("xtv", [B, DM, S], bf16, kind="ExternalInput")
    wq = nc.dram_tensor("wq", [DM, 128], bf16, kind="ExternalInput")
    wk = nc.dram_tensor("wk", [DM, 128], bf16, kind="ExternalInput")
    wv = nc.dram_tensor("wv", [DM, 128], bf16, kind="ExternalInput")
    wo = nc.dram_tensor("wo", [128, DM], f32, kind="ExternalInput")
    masks = nc.dram_tensor("masks", [4, 128, 512], bf16, kind="ExternalInput")
    y = nc.dram_tensor("y", [S, B, DM], bf16, kind="ExternalOutput")

    NJ = DM // 128  # 8 contraction chunks
    NC_Q = S // 512  # 4 s_q chunks per batch
    NT = S // 128  # 16 s_k tiles per batch
    VW = 130  # per-s_k-tile v storage: [v_h0(64) | 1 | v_h1(64) | 1]

    with tile.TileContext(nc) as tc:
        with (
            tc.tile_pool(name="const", bufs=1) as const,
            tc.tile_pool(name="xt", bufs=3) as xtp,
            tc.tile_pool(name="qkv", bufs=1) as qkvp,
            tc.tile_pool(name="pt", bufs=3) as ptp,
            tc.tile_pool(name="osb", bufs=1) as osbp,
            tc.tile_pool(name="sm", bufs=2) as smp,
            tc.tile_pool(name="ps", bufs=2, space="PSUM") as psp,
            tc.tile_pool(name="psacc", bufs=2, space="PSUM") as psaccp,
            tc.tile_pool(name="psr", bufs=2, space="PSUM") as psrp,
        ):
            # ---- constants ----
            wq_sb = const.tile([128, DM], bf16)
            wk_sb = const.tile([128, DM], bf16)
            wv_sb = const.tile([128, DM], bf16)
            wo_sb = const.tile([128, DM], f32)
            masks_sb = const.tile([128, 4 * 512], bf16)
            ones_sb = const.tile([1, 64], f32)
            nc.sync.dma_start(out=wq_sb.rearrange("p (j c) -> p j c", c=128),
                              in_=wq[:, :].rearrange("(j p) c -> p j c", p=128))
            nc.sync.dma_start(out=wk_sb.rearrange("p (j c) -> p j c", c=128),
                              in_=wk[:, :].rearrange("(j p) c -> p j c", p=128))
            nc.sync.dma_start(out=wv_sb.rearrange("p (j c) -> p j c", c=128),
                              in_=wv[:, :].rearrange("(j p) c -> p j c", p=128))
            nc.sync.dma_start(out=wo_sb[:], in_=wo[:, :])
            nc.sync.dma_start(out=masks_sb.rearrange("p (d q) -> p d q", q=512),
                              in_=masks[:, :, :].rearrange("d p q -> p d q"))
            nc.vector.memset(ones_sb[:], 1.0)

            for b in range(B):
                # ---------- Phase A: projections for batch b ----------
                qT_sb = qkvp.tile([128, S], bf16, tag="qT")
                kT_sb = qkvp.tile([128, S], bf16, tag="kT")
                v_sb = qkvp.tile([128, NT * VW], bf16, tag="v")
                for c in range(NC_Q):
                    xq = xtp.tile([128, NJ * 512], bf16, tag="xq")
                    xk = xtp.tile([128, NJ * 512], bf16, tag="xk")
                    xv = xtp.tile([128, NJ * 512], bf16, tag="xv")
                    for xt_sb, xt_dram in ((xq, xtq), (xk, xtk), (xv, xtv)):
                        ov = xt_sb.rearrange("p (j s) -> p j s", s=512)
                        iv = xt_dram[b].rearrange("(j p) s -> p j s", p=128)
                        for g in range(4):  # 4 DMAs/input -> 12 queues busy
                            nc.sync.dma_start(
                                out=ov[:, 2 * g:2 * g + 2, :],
                                in_=iv[:, 2 * g:2 * g + 2,
                                       c * 512:(c + 1) * 512])
                    # qT / kT chunks: out [128(2h x dk), 512]
                    ps_q = psp.tile([128, 512], f32, tag="big")
                    for j in range(NJ):
                        nc.tensor.matmul(ps_q[:], wq_sb[:, j * 128:(j + 1) * 128],
                                         xq[:, j * 512:(j + 1) * 512],
                                         start=(j == 0), stop=(j == NJ - 1))
                    nc.vector.tensor_copy(qT_sb[:, c * 512:(c + 1) * 512], ps_q[:])
                    ps_k = psp.tile([128, 512], f32, tag="big")
                    for j in range(NJ):
                        nc.tensor.matmul(ps_k[:], wk_sb[:, j * 128:(j + 1) * 128],
                                         xk[:, j * 512:(j + 1) * 512],
                                         start=(j == 0), stop=(j == NJ - 1))
                    nc.vector.tensor_copy(kT_sb[:, c * 512:(c + 1) * 512], ps_k[:])
                    # v tiles: out [s 128, 128(2h x dv)]
                    for u in range(4):
                        t = c * 4 + u
                        ps_v = psp.tile([128, 128], f32, tag="big")
                        for j in range(NJ):
                            nc.tensor.matmul(
                                ps_v[:],
                                xv[:, j * 512 + u * 128: j * 512 + (u + 1) * 128],
                                wv_sb[:, j * 128:(j + 1) * 128],
                                start=(j == 0), stop=(j == NJ - 1))
                        nc.vector.tensor_copy(v_sb[:, t * VW:t * VW + 64],
                                              ps_v[:, 0:64])
                        nc.vector.tensor_copy(v_sb[:, t * VW + 65:t * VW + 129],
                                              ps_v[:, 64:128])
                        nc.vector.memset(v_sb[:, t * VW + 64:t * VW + 65], 1.0)
                        nc.vector.memset(v_sb[:, t * VW + 129:t * VW + 130], 1.0)

                # ---------- Phase B: attention ----------
                oT_sb = osbp.tile([128, S], f32, tag="oT")
                for h in range(HEADS_PER_CORE):
                    hp = h * 64
                    for c in range(NC_Q):
                        ot = psaccp.tile([128, 512], f32, tag="acc")
                        n_t = 4 * c + 4
                        for tp0 in range(0, n_t, 2):
                            st = psp.tile([128, 1024], f32, tag="big")
                            for d in range(2):
                                t = tp0 + d
                                nc.tensor.matmul(
                                    st[:, d * 512:(d + 1) * 512],
                                    kT_sb[hp:hp + 64, t * 128:(t + 1) * 128],
                                    qT_sb[hp:hp + 64, c * 512:(c + 1) * 512],
                                    start=True, stop=True)
                            pt = ptp.tile([128, 1024], bf16, tag="pt")
                            nc.scalar.activation(
                                pt[:], st[:],
                                mybir.ActivationFunctionType.Exp, scale=float(SCALE))
                            for d in range(2):
                                t = tp0 + d
                                if t >= 4 * c:  # diagonal tile -> causal mask
                                    dd = t - 4 * c
                                    nc.vector.tensor_mul(
                                        pt[:, d * 512:(d + 1) * 512],
                                        pt[:, d * 512:(d + 1) * 512],
                                        masks_sb[:, dd * 512:(dd + 1) * 512])
                            for d in range(2):
                                t = tp0 + d
                                nc.tensor.matmul(
                                    ot[0:65, :],
                                    v_sb[:, t * VW + h * 65:t * VW + h * 65 + 65],
                                    pt[:, d * 512:(d + 1) * 512],
                                    start=(t == 0), stop=(t == n_t - 1))
                        # normalize: rows 0:64 /= row 64
                        rsum = smp.tile([1, 512], f32, tag="rsum")
                        nc.vector.tensor_copy(rsum[:], ot[64:65, :])
                        rps = psrp.tile([64, 512], f32, tag="r")
                        nc.tensor.matmul(rps[:], ones_sb[:], rsum[:],
                                         start=True, stop=True)
                        rcp = smp.tile([64, 512], f32, tag="rcp")
                        nc.vector.reciprocal(rcp[:], rps[:])
                        nc.vector.tensor_mul(
                            oT_sb[hp:hp + 64, c * 512:(c + 1) * 512],
                            ot[0:64, :], rcp[:])

                # ---------- Phase C: y = oT.T @ WO_local ----------
                for t in range(NT):
                    for m in range(DM // 512):
                        yps = psp.tile([128, 512], f32, tag="big")
                        nc.tensor.matmul(yps[:],
                                         oT_sb[:, t * 128:(t + 1) * 128],
                                         wo_sb[:, m * 512:(m + 1) * 512],
                                         start=True, stop=True)
                        ysb = ptp.tile([128, 512], bf16, tag="ysb")
                        nc.vector.tensor_copy(ysb[:], yps[:])
                        nc.sync.dma_start(
                            out=y[t * 128:(t + 1) * 128, b, m * 512:(m + 1) * 512],
                            in_=ysb[:])
    if split_waits:
        _split_waits(nc, mybir)
    return nc


def _split_waits(nc, mybir):
    """This walrus build encodes at most one sync wait per instruction
    (fp32 Matmult LDW slots overflow at two).  Hoist every wait onto its
    own same-engine NoOp issued immediately before the instruction —
    semantically identical: the sequencer blocks at the NoOp instead."""
    ctr = [0]
    for fn in nc.m.functions:
        for blk in fn.blocks:
            new_insts = []
            for ins in blk.instructions:
                si = getattr(ins, "sync_info", None)
                waits = list(si.on_wait) if si is not None and si.on_wait else []
                if waits:
                    for w in waits:
                        ctr[0] += 1
                        nop = mybir.InstNoOp(name=f"WSPLIT-{ctr[0]}", ins=[], outs=[])
                        nop.engine = ins.engine
                        nop.sync_info = mybir.SyncInfo(on_wait=[w], on_update=[])
                        new_insts.append(nop)
                    ins.sync_info = mybir.SyncInfo(
                        on_wait=[], on_update=list(si.on_update or []))
                new_insts.append(ins)
            blk.instructions = new_insts


def _marshal(Q, K, V, WQ, WK, WV, WO):
    Q = np.asarray(Q, dtype=np.float32)
    K = np.asarray(K, dtype=np.float32)
    V = np.asarray(V, dtype=np.float32)
    WQ = np.asarray(WQ, dtype=np.float32)
    WK = np.asarray(WK, dtype=np.float32)
    WV = np.asarray(WV, dtype=np.float32)
    WO = np.asarray(WO, dtype=np.float32)

    import ml_dtypes
    bf = ml_dtypes.bfloat16
    xtq = np.ascontiguousarray(Q.transpose(1, 2, 0)).astype(bf)  # [B, DM, S]
    xtk = np.ascontiguousarray(K.transpose(1, 2, 0)).astype(bf)
    xtv = np.ascontiguousarray(V.transpose(1, 2, 0)).astype(bf)

    masks = np.zeros((4, 128, 512), dtype=bf)
    kk = np.arange(128)[:, None]
    qq = np.arange(512)[None, :]
    for d in range(4):
        masks[d] = (d * 128 + kk <= qq).astype(bf)

    in_maps = []
    for core in range(NCORES):
        h0 = core * HEADS_PER_CORE
        wql = np.ascontiguousarray(np.concatenate([WQ[h0], WQ[h0 + 1]], axis=1)).astype(bf)
        wkl = np.ascontiguousarray(np.concatenate([WK[h0], WK[h0 + 1]], axis=1)).astype(bf)
        wvl = np.ascontiguousarray(np.concatenate([WV[h0], WV[h0 + 1]], axis=1)).astype(bf)
        wol = np.ascontiguousarray(WO[h0 * DV:(h0 + 2) * DV, :])
        in_maps.append({
            "xtq": xtq, "xtk": xtk, "xtv": xtv,
            "wq": wql, "wk": wkl, "wv": wvl, "wo": wol,
            "masks": masks,
        })
    return in_maps


LAST_RESULTS = None


def kernel(Q, K, V, WQ, WK, WV, WO):
    global LAST_RESULTS
    from concourse.bass_utils import run_bass_kernel_spmd

    if "nc" not in _CACHE:
        _CACHE["nc"] = build_nc()
    nc = _CACHE["nc"]

    in_maps = _marshal(Q, K, V, WQ, WK, WV, WO)
    res = run_bass_kernel_spmd(nc, in_maps, core_ids=list(range(NCORES)))
    LAST_RESULTS = res
    out = np.zeros((S, B, DM), dtype=np.float32)
    for r in res.results:
        out += np.asarray(r["y"]).astype(np.float32)
    return out



# revision 1
# speedup vs baseline: 2.5334x; 2.5334x over previous
"""Multi-head attention (16 heads, DM=1024, DK=DV=64, S=2048, B=2, causal)
tensor-parallel over heads on 8 NeuronCores (2 heads per core).

Host-side marshalling:
  - Activations are pre-transposed to XT[B, DM, S] so the device matmuls
    contract over DM on the partition dimension with natural layouts.
  - Per-core weights: WQ/WK/WV head pair stacked on columns -> [DM, 128];
    WO rows for the head pair -> [128, DM].
  - Each core computes its 2 heads end-to-end plus the WO partial
    projection; host sums the 8 partial outputs.

Device layouts (per batch b):
  qT, kT  [128(2 heads x dk), S]   = (W.T @ XT) via lhsT=W chunk, rhs=XT chunk
  v       [S, 128(2 heads x dv)]   interleaved with ones cols for row-sums
  ST      [s_k 128, s_q 512]       = kT_tile.T-style scores, transposed
  pT      = exp(ST/8) (no max subtraction needed: |scores/8| <= ~2)
  oT      [65, s_q 512] accum      rows 0:64 = unnormalized head out, row 64 = rowsum
  y       [s 128, dm 512]          = oT.T @ WO_local, accumulated over both heads via K=128
"""

import numpy as np

S, B, DM, DK, DV, H = 2048, 2, 1024, 64, 64, 16
NCORES = 8
HEADS_PER_CORE = H // NCORES  # 2
SCALE = 1.0 / np.sqrt(DK)  # 1/8

F32 = None  # filled lazily (mybir import)

_CACHE = {}


def build_nc(split_waits=True):
    import concourse.bass as bass
    import concourse.tile as tile
    from concourse import mybir

    f32 = mybir.dt.float32
    bf16 = mybir.dt.bfloat16
    nc = bass.Bass()

    xtq = nc.dram_tensor("xtq", [B, DM, S], bf16, kind="ExternalInput")
    xtk = nc.dram_tensor("xtk", [B, DM, S], bf16, kind="ExternalInput")
    xtv = nc.dram_tensor("xtv", [B, DM, S], bf16, kind="ExternalInput")
    wq = nc.dram_tensor("wq", [DM, 128], bf16, kind="ExternalInput")
    wk = nc.dram_tensor("wk", [DM, 128], bf16, kind="ExternalInput")
    wv = nc.dram_tensor("wv", [DM, 128], bf16, kind="ExternalInput")
    wo = nc.dram_tensor("wo", [128, DM], f32, kind="ExternalInput")
    masks = nc.dram_tensor("masks", [4, 128, 512], bf16, kind="ExternalInput")
    y = nc.dram_tensor("y", [S, B, DM], bf16, kind="ExternalOutput")

    NJ = DM // 128  # 8 contraction chunks
    NC_Q = S // 512  # 4 s_q chunks per batch
    NT = S // 128  # 16 s_k tiles per batch
    VW = 130  # per-s_k-tile v storage: [v_h0(64) | 1 | v_h1(64) | 1]

    with tile.TileContext(nc) as tc:
        with (
            tc.tile_pool(name="const", bufs=1) as const,
            tc.tile_pool(name="xt", bufs=3) as xtp,
            tc.tile_pool(name="qkv", bufs=1) as qkvp,
            tc.tile_pool(name="pt", bufs=3) as ptp,
            tc.tile_pool(name="osb", bufs=1) as osbp,
            tc.tile_pool(name="sm", bufs=2) as smp,
            tc.tile_pool(name="ps", bufs=2, space="PSUM") as psp,
            tc.tile_pool(name="psacc", bufs=2, space="PSUM") as psaccp,
            tc.tile_pool(name="psr", bufs=2, space="PSUM") as psrp,
        ):
            # ---- constants ----
            wq_sb = const.tile([128, DM], bf16)
            wk_sb = const.tile([128, DM], bf16)
            wv_sb = const.tile([128, DM], bf16)
            wo_sb = const.tile([128, DM], f32)
            masks_sb = const.tile([128, 4 * 512], bf16)
            ones_sb = const.tile([1, 64], f32)
            nc.sync.dma_start(out=wq_sb.rearrange("p (j c) -> p j c", c=128),
                              in_=wq[:, :].rearrange("(j p) c -> p j c", p=128))
            nc.sync.dma_start(out=wk_sb.rearrange("p (j c) -> p j c", c=128),
                              in_=wk[:, :].rearrange("(j p) c -> p j c", p=128))
            nc.sync.dma_start(out=wv_sb.rearrange("p (j c) -> p j c", c=128),
                              in_=wv[:, :].rearrange("(j p) c -> p j c", p=128))
            nc.sync.dma_start(out=wo_sb[:], in_=wo[:, :])
            nc.sync.dma_start(out=masks_sb.rearrange("p (d q) -> p d q", q=512),
                              in_=masks[:, :, :].rearrange("d p q -> p d q"))
            nc.vector.memset(ones_sb[:], 1.0)

            for b in range(B):
                # ---------- Phase A: projections for batch b ----------
                qT_sb = qkvp.tile([128, S], bf16, tag="qT")
                kT_sb = qkvp.tile([128, S], bf16, tag="kT")
                v_sb = qkvp.tile([128, NT * VW], bf16, tag="v")
                for c in range(NC_Q):
                    xq = xtp.tile([128, NJ * 512], bf16, tag="xq")
                    xk = xtp.tile([128, NJ * 512], bf16, tag="xk")
                    xv = xtp.tile([128, NJ * 512], bf16, tag="xv")
                    for xt_sb, xt_dram in ((xq, xtq), (xk, xtk), (xv, xtv)):
                        ov = xt_sb.rearrange("p (j s) -> p j s", s=512)
                        iv = xt_dram[b].rearrange("(j p) s -> p j s", p=128)
                        for g in range(4):  # 4 DMAs/input -> 12 queues busy
                            nc.sync.dma_start(
                                out=ov[:, 2 * g:2 * g + 2, :],
                                in_=iv[:, 2 * g:2 * g + 2,
                                       c * 512:(c + 1) * 512])
                    # qT / kT chunks: out [128(2h x dk), 512]
                    ps_q = psp.tile([128, 512], f32, tag="big")
                    for j in range(NJ):
                        nc.tensor.matmul(ps_q[:], wq_sb[:, j * 128:(j + 1) * 128],
                                         xq[:, j * 512:(j + 1) * 512],
                                         start=(j == 0), stop=(j == NJ - 1))
                    nc.vector.tensor_copy(qT_sb[:, c * 512:(c + 1) * 512], ps_q[:])
                    ps_k = psp.tile([128, 512], f32, tag="big")
                    for j in range(NJ):
                        nc.tensor.matmul(ps_k[:], wk_sb[:, j * 128:(j + 1) * 128],
                                         xk[:, j * 512:(j + 1) * 512],
                                         start=(j == 0), stop=(j == NJ - 1))
                    nc.vector.tensor_copy(kT_sb[:, c * 512:(c + 1) * 512], ps_k[:])
                    # v tiles: out [s 128, 128(2h x dv)]
                    for u in range(4):
                        t = c * 4 + u
                        ps_v = psp.tile([128, 128], f32, tag="big")
                        for j in range(NJ):
                            nc.tensor.matmul(
                                ps_v[:],
                                xv[:, j * 512 + u * 128: j * 512 + (u + 1) * 128],
                                wv_sb[:, j * 128:(j + 1) * 128],
                                start=(j == 0), stop=(j == NJ - 1))
                        nc.vector.tensor_copy(v_sb[:, t * VW:t * VW + 64],
                                              ps_v[:, 0:64])
                        nc.vector.tensor_copy(v_sb[:, t * VW + 65:t * VW + 129],
                                              ps_v[:, 64:128])
                        nc.vector.memset(v_sb[:, t * VW + 64:t * VW + 65], 1.0)
                        nc.vector.memset(v_sb[:, t * VW + 129:t * VW + 130], 1.0)

                # ---------- Phase B: attention ----------
                oT_sb = osbp.tile([128, S], f32, tag="oT")
                for h in range(HEADS_PER_CORE):
                    hp = h * 64
                    for c in range(NC_Q):
                        ot = psaccp.tile([128, 512], f32, tag="acc")
                        n_t = 4 * c + 4
                        for tp0 in range(0, n_t, 2):
                            st = psp.tile([128, 1024], f32, tag="big")
                            for d in range(2):
                                t = tp0 + d
                                nc.tensor.matmul(
                                    st[:, d * 512:(d + 1) * 512],
                                    kT_sb[hp:hp + 64, t * 128:(t + 1) * 128],
                                    qT_sb[hp:hp + 64, c * 512:(c + 1) * 512],
                                    start=True, stop=True)
                            pt = ptp.tile([128, 1024], bf16, tag="pt")
                            nc.scalar.activation(
                                pt[:], st[:],
                                mybir.ActivationFunctionType.Exp, scale=float(SCALE))
                            for d in range(2):
                                t = tp0 + d
                                if t >= 4 * c:  # diagonal tile -> causal mask
                                    dd = t - 4 * c
                                    nc.vector.tensor_mul(
                                        pt[:, d * 512:(d + 1) * 512],
                                        pt[:, d * 512:(d + 1) * 512],
                                        masks_sb[:, dd * 512:(dd + 1) * 512])
                            for d in range(2):
                                t = tp0 + d
                                nc.tensor.matmul(
                                    ot[0:65, :],
                                    v_sb[:, t * VW + h * 65:t * VW + h * 65 + 65],
                                    pt[:, d * 512:(d + 1) * 512],
                                    start=(t == 0), stop=(t == n_t - 1))
                        # normalize: rows 0:64 /= row 64
                        rsum = smp.tile([1, 512], f32, tag="rsum")
                        nc.vector.tensor_copy(rsum[:], ot[64:65, :])
                        rps = psrp.tile([64, 512], f32, tag="r")
                        nc.tensor.matmul(rps[:], ones_sb[:], rsum[:],
                                         start=True, stop=True)
                        rcp = smp.tile([64, 512], f32, tag="rcp")
                        nc.vector.reciprocal(rcp[:], rps[:])
                        nc.vector.tensor_mul(
                            oT_sb[hp:hp + 64, c * 512:(c + 1) * 512],
                            ot[0:64, :], rcp[:])

                # ---------- Phase C: y = oT.T @ WO_local ----------
                for t in range(NT):
                    for m in range(DM // 512):
                        yps = psp.tile([128, 512], f32, tag="big")
                        nc.tensor.matmul(yps[:],
                                         oT_sb[:, t * 128:(t + 1) * 128],
                                         wo_sb[:, m * 512:(m + 1) * 512],
                                         start=True, stop=True)
                        ysb = ptp.tile([128, 512], bf16, tag="ysb")
                        nc.vector.tensor_copy(ysb[:], yps[:])
                        nc.sync.dma_start(
                            out=y[t * 128:(t + 1) * 128, b, m * 512:(m + 1) * 512],
                            in_=ysb[:])
    if split_waits:
        _split_waits(nc, mybir)
    return nc


def _split_waits(nc, mybir):
    """This walrus build encodes at most one sync wait per instruction
    (fp32 Matmult LDW slots overflow at two).  Hoist every wait onto its
    own same-engine NoOp issued immediately before the instruction —
    semantically identical: the sequencer blocks at the NoOp instead."""
    ctr = [0]
    for fn in nc.m.functions:
        for blk in fn.blocks:
            new_insts = []
            for ins in blk.instructions:
                si = getattr(ins, "sync_info", None)
                waits = list(si.on_wait) if si is not None and si.on_wait else []
                if waits:
                    for w in waits:
                        ctr[0] += 1
                        nop = mybir.InstNoOp(name=f"WSPLIT-{ctr[0]}", ins=[], outs=[])
                        nop.engine = ins.engine
                        nop.sync_info = mybir.SyncInfo(on_wait=[w], on_update=[])
                        new_insts.append(nop)
                    ins.sync_info = mybir.SyncInfo(
                        on_wait=[], on_update=list(si.on_update or []))
                new_insts.append(ins)
            blk.instructions = new_insts


def _marshal(Q, K, V, WQ, WK, WV, WO):
    Q = np.asarray(Q, dtype=np.float32)
    K = np.asarray(K, dtype=np.float32)
    V = np.asarray(V, dtype=np.float32)
    WQ = np.asarray(WQ, dtype=np.float32)
    WK = np.asarray(WK, dtype=np.float32)
    WV = np.asarray(WV, dtype=np.float32)
    WO = np.asarray(WO, dtype=np.float32)

    import ml_dtypes
    bf = ml_dtypes.bfloat16
    xtq = np.ascontiguousarray(Q.transpose(1, 2, 0)).astype(bf)  # [B, DM, S]
    xtk = np.ascontiguousarray(K.transpose(1, 2, 0)).astype(bf)
    xtv = np.ascontiguousarray(V.transpose(1, 2, 0)).astype(bf)

    masks = np.zeros((4, 128, 512), dtype=bf)
    kk = np.arange(128)[:, None]
    qq = np.arange(512)[None, :]
    for d in range(4):
        masks[d] = (d * 128 + kk <= qq).astype(bf)

    in_maps = []
    for core in range(NCORES):
        h0 = core * HEADS_PER_CORE
        wql = np.ascontiguousarray(np.concatenate([WQ[h0], WQ[h0 + 1]], axis=1)).astype(bf)
        wkl = np.ascontiguousarray(np.concatenate([WK[h0], WK[h0 + 1]], axis=1)).astype(bf)
        wvl = np.ascontiguousarray(np.concatenate([WV[h0], WV[h0 + 1]], axis=1)).astype(bf)
        wol = np.ascontiguousarray(WO[h0 * DV:(h0 + 2) * DV, :])
        in_maps.append({
            "xtq": xtq, "xtk": xtk, "xtv": xtv,
            "wq": wql, "wk": wkl, "wv": wvl, "wo": wol,
            "masks": masks,
        })
    return in_maps


LAST_RESULTS = None


def kernel(Q, K, V, WQ, WK, WV, WO):
    global LAST_RESULTS
    from concourse.bass_utils import run_bass_kernel_spmd

    if "nc" not in _CACHE:
        _CACHE["nc"] = build_nc()
    nc = _CACHE["nc"]

    in_maps = _marshal(Q, K, V, WQ, WK, WV, WO)
    res = run_bass_kernel_spmd(nc, in_maps, core_ids=list(range(NCORES)))
    LAST_RESULTS = res
    out = np.zeros((S, B, DM), dtype=np.float32)
    for r in res.results:
        out += np.asarray(r["y"]).astype(np.float32)
    return out

